# revision 24
# baseline (speedup 1.0000x reference)
"""Windowed attention block (LeViT-style) on 8 Trainium2 NeuronCores.

LayerNorm -> QKV -> per-head biased softmax attention -> output projection
for B=256 windows, N=196 tokens, DIM=384, 12 heads of dim 32.

Sharding: data-parallel over the window dim B — 32 windows per core, weights
replicated, no collectives. Each core runs an identical Bass/Tile program on
its shard; the host concatenates the 8 output shards.

Kernel strategy (per window):
 - LN token-major via bn_stats + exp(-0.5*ln(var+eps)), applied on ScalarE
   (norm_w/norm_b are folded into the QKV weights on the host).
 - xhat transposed to feature-major via PE transpose (bf16).
 - QKV computed feature-major for q,k (so per-head qT/kT are direct slices at
   32-aligned partitions) and token-major for v (AV's lhsT layout).
 - scoresT[h] = kT_h^T @ qT_h with 4 heads running concurrently in distinct
   32-row PE groups (tile_position row tiling, K=32).
 - probs = exp(scoresT) * exp(bias)  (bias table gathered+exp'd on host; the
   multiplicative form lets ScalarE do exp straight out of PSUM while
   VectorE/GpSimd apply the bias in bf16).
 - AV col-tiled: per head pair, v-columns at PE col groups 0-1 and a [128,32]
   ones block at col groups 2-3, so one PSUM bank accumulates both heads'
   outputs (rows 0:64) and their softmax sums replicated 32x (rows 64:128).
   The replicated sums make 1/sum a contiguous [64,196] reciprocal whose
   output needs no broadcast for the normalize multiply.
 - proj from the normalized feature-major outT, fp32 result copied and DMA'd.

Token dim padded 196->256 (zero k-columns + zero ebias rows) so both tok_k
chunks use all 128 partitions.
"""

import os
import sys
import numpy as np

sys.path.insert(0, "/root/.axon_site/_ro/trn_rl_repo")

import ml_dtypes

B, N, DIM = 256, 196, 384
H, KD, VD = 12, 32, 32
RES = 14
EPS = 1e-5
NCORES = 8
WPC = B // NCORES          # windows per core
NP = 256                   # padded token count (2 chunks of 128)
NB = N - 128               # 68 = second token chunk size

BF16 = ml_dtypes.bfloat16


def _build_bias_idxs():
    pts = [(i, j) for i in range(RES) for j in range(RES)]
    offs, idxs = {}, []
    for p1 in pts:
        for p2 in pts:
            o = (abs(p1[0] - p2[0]), abs(p1[1] - p2[1]))
            if o not in offs:
                offs[o] = len(offs)
            idxs.append(offs[o])
    return np.array(idxs, dtype=np.int32).reshape(N, N)


def _split_waits(nc, keep=1):
    """Hoist excess sem-waits into standalone single-wait NoOps.

    The walrus build here rejects instructions whose sync region carries more
    than ~2 sync commands; Tile attaches every required wait directly to the
    instruction (and its tail drain waits on every live proc). A chain of
    single-wait NoOps on the same engine immediately before the instruction
    is semantically identical (the engine's instruction stream blocks), so
    this rewrite preserves correctness.
    """
    from concourse import mybir
    counter = [0]

    def fresh():
        counter[0] += 1
        return f"I-waitsplit-{counter[0]}"

    for f in nc.m.functions:
        for blk in f.blocks:
            out, changed = [], False
            for inst in blk.instructions:
                si = inst.sync_info
                waits = list(si.on_wait) if si is not None and si.on_wait else []
                if len(waits) > keep:
                    changed = True
                    for wt in waits[:-keep]:
                        nop = mybir.InstNoOp(name=fresh(), ins=[], outs=[])
                        nop.engine = inst.engine
                        nop.sync_info = mybir.SyncInfo(on_wait=[wt], on_update=[])
                        out.append(nop)
                    inst.sync_info = mybir.SyncInfo(
                        on_wait=waits[-keep:],
                        on_update=list(si.on_update) if si.on_update else [])
                out.append(inst)
            if changed:
                blk.instructions = out


def _build_program(has_qk_bias, has_v_bias):
    import concourse.bass as bass
    import concourse.tile as tile
    from concourse import mybir

    F32 = mybir.dt.float32
    BF = mybir.dt.bfloat16
    AF = mybir.ActivationFunctionType
    ALU = mybir.AluOpType

    nc = bass.Bass()
    # Pre-register an eps const AP so `activation(..., bias=EPS)` carries no
    # runtime dependency (mirrors Bass's own const-AP registration).
    _epsc = nc.alloc_sbuf_tensor("const-eps", [128, 1], F32)
    nc.gpsimd.memset(_epsc.ap(), EPS)
    nc.const_aps.aps[(F32, float(EPS))] = _epsc.ap()
    nc.all_engine_barrier()
    # x is stored flat with 64 rows of zero padding so each window can be
    # fetched as one [128, 768] interleaved DMA (token t -> partition t%128,
    # column block t//128).
    x_d = nc.dram_tensor("x", [WPC * N + 64, DIM], F32, kind="ExternalInput")
    wqk_d = nc.dram_tensor("wqk", [DIM, 2 * DIM], BF, kind="ExternalInput")
    wv_d = nc.dram_tensor("wv", [DIM, DIM], BF, kind="ExternalInput")
    wp_d = nc.dram_tensor("wp", [DIM, DIM], BF, kind="ExternalInput")
    # raw (transposed) attention bias per head, preloaded into the scores
    # PSUM bank via an identity matmul before the score matmuls accumulate.
    eb_d = nc.dram_tensor("eb", [128, H * 392], BF, kind="ExternalInput")
    id_d = nc.dram_tensor("idm", [128, 128], BF, kind="ExternalInput")
    if has_qk_bias:
        qkb_d = nc.dram_tensor("qkb", [2 * DIM, 1], F32, kind="ExternalInput")
    if has_v_bias:
        vb_d = nc.dram_tensor("vb", [128, DIM], F32, kind="ExternalInput")
    out_d = nc.dram_tensor("out", [WPC, N, DIM], F32, kind="ExternalOutput")

    with tile.TileContext(nc) as tc:
        with tc.tile_pool(name="const", bufs=1) as cp, \
             tc.tile_pool(name="work", bufs=2) as wk, \
             tc.tile_pool(name="ps", bufs=8, space="PSUM") as ps:

            # ---- persistent constants ----
            wqk_sb = []
            for i in range(3):
                t = cp.tile([128, 2 * DIM], BF, name=f"wqk{i}")
                nc.sync.dma_start(t, wqk_d.ap()[128 * i:128 * (i + 1), :])
                wqk_sb.append(t)
            wv_sb = []
            for i in range(3):
                t = cp.tile([128, DIM], BF, name=f"wv{i}")
                nc.sync.dma_start(t, wv_d.ap()[128 * i:128 * (i + 1), :])
                wv_sb.append(t)
            wp_sb = []
            for i in range(3):
                t = cp.tile([128, DIM], BF, name=f"wp{i}")
                nc.sync.dma_start(t, wp_d.ap()[128 * i:128 * (i + 1), :])
                wp_sb.append(t)
            eb_sb = cp.tile([128, H * 392], BF, name="ebias")
            nc.sync.dma_start(eb_sb, eb_d.ap())
            ident = cp.tile([128, 128], BF, name="ident")
            nc.sync.dma_start(ident, id_d.ap())
            ones_sb = cp.tile([128, 32], BF, name="ones32")
            nc.gpsimd.memset(ones_sb, 1.0)
            if has_qk_bias:
                qkb_sb = []
                for i in range(6):
                    t = cp.tile([128, 1], F32, name=f"qkb{i}")
                    nc.sync.dma_start(t, qkb_d.ap()[128 * i:128 * (i + 1), :])
                    qkb_sb.append(t)
            if has_v_bias:
                vb_sb = cp.tile([128, DIM], F32, name="vbias")
                nc.sync.dma_start(vb_sb, vb_d.ap())

            for wp in range(WPC // 2):
                # Two windows are processed jointly through the feature-major
                # stages (transpose/q/k), doubling those tiles' free dim to
                # 392 and halving the op count on the copy-bound engines.
                toks = [(0, 128), (128, NB)]
                wins = (2 * wp, 2 * wp + 1)
                xh_w = []
                v_w = []
                for sub, w in enumerate(wins):
                    # ---- load x: one interleaved DMA [128, 768] ----
                    x_sb = wk.tile([128, 2 * DIM], F32, name=f"x{sub}")
                    nc.sync.dma_start(
                        x_sb,
                        bass.AP(x_d, w * N * DIM,
                                [[DIM, 128], [128 * DIM, 2], [1, DIM]]))

                    # ---- LayerNorm (stats DVE, ln/exp ACT, apply GpSimd) ----
                    xh_t = []
                    for ci, (t0, tn) in enumerate(toks):
                        xc = x_sb[:, DIM * ci:DIM * (ci + 1)]
                        bn6 = wk.tile([128, 6], F32, name=f"bn{sub}{ci}")
                        nc.vector.bn_stats(bn6, xc)
                        mv = wk.tile([128, 2], F32, name=f"mv{sub}{ci}")
                        nc.vector.bn_aggr(mv, bn6)
                        lnv = wk.tile([128, 1], F32, name=f"lnv{sub}{ci}")
                        nc.scalar.activation(lnv, mv[:, 1:2], AF.Ln, bias=EPS)
                        rstd = wk.tile([128, 1], F32, name=f"rstd{sub}{ci}")
                        nc.scalar.activation(rstd, lnv, AF.Exp, scale=-0.5)
                        nrstd = wk.tile([128, 1], F32, name=f"nrstd{sub}{ci}")
                        nc.scalar.activation(nrstd, rstd, AF.Identity, scale=-1.0)
                        nmr = wk.tile([128, 1], F32, name=f"nmr{sub}{ci}")
                        nc.scalar.activation(nmr, mv[:, 0:1], AF.Identity,
                                             scale=nrstd[:, 0:1])
                        xh = wk.tile([128, DIM], BF, name=f"xh{sub}{ci}")
                        nc.gpsimd.tensor_scalar(xh, xc, rstd[:, 0:1], nmr[:, 0:1],
                                                ALU.mult, ALU.add)
                        xh_t.append(xh)
                    xh_w.append(xh_t)

                # ---- transpose xhat -> feature-major [384, 392] (3 tiles) ----
                xT = []
                for i in range(3):
                    xT_ps = ps.tile([128, 2 * N], BF, name="xTps", tag="bank")
                    for sub in range(2):
                        o = N * sub
                        nc.tensor.transpose(xT_ps[:, o:o + 128],
                                            xh_w[sub][0][:, 128 * i:128 * (i + 1)],
                                            ident)
                        nc.tensor.transpose(xT_ps[:, o + 128:o + N],
                                            xh_w[sub][1][0:NB, 128 * i:128 * (i + 1)],
                                            ident[0:NB, 0:NB])
                    xTs = wk.tile([128, 2 * N], BF, name=f"xT{i}", bufs=6)
                    nc.vector.tensor_copy(xTs, xT_ps)
                    xT.append(xTs)

                # ---- q,k feature-major for both windows ----
                q_sb, k_sb = [], []
                for i in range(3):
                    qp = ps.tile([128, 2 * N], F32, name="qps", tag="bank")
                    for d in range(3):
                        nc.tensor.matmul(qp, wqk_sb[d][:, 128 * i:128 * (i + 1)],
                                         xT[d], start=(d == 0), stop=(d == 2))
                    qs = wk.tile([128, 2 * N], BF, name=f"q{i}", bufs=6)
                    if has_qk_bias:
                        nc.scalar.activation(qs, qp, AF.Identity,
                                             bias=qkb_sb[i][:, 0:1])
                    else:
                        nc.scalar.copy(qs, qp)
                    q_sb.append(qs)
                for i in range(3):
                    kp = ps.tile([128, 2 * N], F32, name="kps", tag="bank")
                    for d in range(3):
                        nc.tensor.matmul(kp, wqk_sb[d][:, DIM + 128 * i:DIM + 128 * (i + 1)],
                                         xT[d], start=(d == 0), stop=(d == 2))
                    # layout [w0 196 | pad 60 | w1 196 | pad 60]
                    ks = wk.tile([128, 2 * NP], BF, name=f"k{i}", bufs=6)
                    nc.gpsimd.memset(
                        bass.AP(ks.tensor, ks.offset + N,
                                [list(ks.ap[0]), [NP, 2], [1, NP - N]]), 0.0)
                    dst = bass.AP(ks.tensor, ks.offset,
                                  [list(ks.ap[0]), [NP, 2], [1, N]])
                    if has_qk_bias:
                        nc.scalar.activation(dst, kp, AF.Identity,
                                             bias=qkb_sb[3 + i][:, 0:1])
                    else:
                        nc.vector.tensor_copy(dst, kp)
                    k_sb.append(ks)

                # ---- v token-major [256(pad), 384] per window ----
                for sub in range(2):
                    v_sb = []
                    for ci, (t0, tn) in enumerate(toks):
                        vp = ps.tile([128, DIM], F32, name="vps", tag="bank")
                        for d in range(3):
                            nc.tensor.matmul(vp[0:tn, :],
                                             xT[d][:, N * sub + t0:N * sub + t0 + tn],
                                             wv_sb[d], start=(d == 0), stop=(d == 2))
                        vs = wk.tile([128, DIM], BF, name=f"v{sub}{ci}", bufs=4)
                        if tn < 128:
                            # pad rows zeroed; the copy below rewrites real
                            # rows 64:tn (Tile orders the overlapping writes)
                            nc.gpsimd.memset(vs[64:128, :], 0.0)
                        nc.vector.tensor_copy(vs[0:tn, :], vp[0:tn, :])
                        if has_v_bias:
                            nc.vector.tensor_tensor(vs[0:tn, :], vs[0:tn, :],
                                                    vb_sb[0:tn, :], ALU.add)
                        v_sb.append(vs)
                    v_w.append(v_sb)

                for sub, w in enumerate(wins):
                    qo, ko = N * sub, NP * sub
                    v_sb = v_w[sub]
                    # ---- scoresT + probs per head ----
                    # The (transposed) attention bias is preloaded into the
                    # scores PSUM bank by an identity matmul; the two K=32
                    # row-tiled score matmuls accumulate on top, so exp(PSUM)
                    # IS the probability tile (padded keys: bias -30 -> ~0).
                    probs2 = [None] * H
                    for h in range(H):
                        g, r = h // 4, (h % 4) * 32
                        sp = ps.tile([128, 392], F32, name="scp", tag="bank")
                        nc.tensor.matmul(sp, ident,
                                         eb_sb[:, 392 * h:392 * (h + 1)],
                                         start=True, stop=False,
                                         skip_group_check=True)
                        nc.tensor.matmul(sp[:, 0:N],
                                         k_sb[g][r:r + 32, ko:ko + 128],
                                         q_sb[g][r:r + 32, qo:qo + N],
                                         tile_position=(r, 0),
                                         start=False, stop=False,
                                         skip_group_check=True)
                        nc.tensor.matmul(sp[:, 196:196 + N],
                                         k_sb[g][r:r + 32, ko + 128:ko + NP],
                                         q_sb[g][r:r + 32, qo:qo + N],
                                         tile_position=(r, 0),
                                         start=False, stop=True,
                                         skip_group_check=True)
                        pr = wk.tile([128, 392], BF, name="probs", bufs=14)
                        nc.scalar.activation(pr, sp, AF.Exp)
                        probs2[h] = pr

                    # ---- AV + sums per pair; reciprocal of replicated sums ----
                    av_ps, rc_sb = [], []
                    for p in range(6):
                        h0, h1 = 2 * p, 2 * p + 1
                        ap_ = ps.tile([128, N], F32, name="avp", tag="bank")
                        for c in range(2):
                            st, fi = (c == 0), (c == 1)
                            pa = probs2[h0][:, 196 * c:196 * (c + 1)]
                            pb = probs2[h1][:, 196 * c:196 * (c + 1)]
                            nc.tensor.matmul(ap_[0:32, :],
                                             v_sb[c][:, 32 * h0:32 * h0 + 32],
                                             pa, start=st, stop=fi,
                                             tile_position=(0, 0))
                            nc.tensor.matmul(ap_[32:64, :],
                                             v_sb[c][:, 32 * h1:32 * h1 + 32],
                                             pb, start=st, stop=fi,
                                             tile_position=(0, 32))
                            nc.tensor.matmul(ap_[64:96, :], ones_sb, pa,
                                             start=st, stop=fi,
                                             tile_position=(0, 64))
                            nc.tensor.matmul(ap_[96:128, :], ones_sb, pb,
                                             start=st, stop=fi,
                                             tile_position=(0, 96))
                        rc = wk.tile([64, N], F32, name="rc", bufs=8)
                        nc.vector.reciprocal(rc, ap_[64:128, :])
                        av_ps.append(ap_)
                        rc_sb.append(rc)

                    # ---- normalize -> outT feature-major [384, 196] bf16 ----
                    oT = []
                    for t in range(3):
                        o = wk.tile([128, N], BF, name=f"oT{t}", bufs=6)
                        nc.vector.tensor_tensor(o[0:64, :], av_ps[2 * t][0:64, :],
                                                rc_sb[2 * t], ALU.mult)
                        nc.vector.tensor_tensor(o[64:128, :],
                                                av_ps[2 * t + 1][0:64, :],
                                                rc_sb[2 * t + 1], ALU.mult)
                        oT.append(o)

                    # ---- projection + store ----
                    for ci, (t0, tn) in enumerate(toks):
                        pp = ps.tile([128, DIM], F32, name="pjps", tag="bank")
                        for t in range(3):
                            nc.tensor.matmul(pp[0:tn, :], oT[t][:, t0:t0 + tn],
                                             wp_sb[t], start=(t == 0), stop=(t == 2))
                        ob = wk.tile([128, DIM], F32, name=f"ob{ci}")
                        nc.scalar.copy(ob[0:tn, :], pp[0:tn, :])
                        # store via gpsimd SWDGE to keep the SP sequencer free
                        nc.gpsimd.dma_start(out_d.ap()[w, t0:t0 + tn, :],
                                            ob[0:tn, :])

    _split_waits(nc)
    return nc


_CACHE = {}


def kernel(x, norm_w, norm_b, qkv_w, qkv_b, attention_biases, proj_w, proj_b,
           bias_idxs):
    x = np.asarray(x, np.float32)
    norm_w = np.asarray(norm_w, np.float32)
    norm_b = np.asarray(norm_b, np.float32)
    qkv_w = np.asarray(qkv_w, np.float32)
    qkv_b = np.asarray(qkv_b, np.float32)
    attention_biases = np.asarray(attention_biases, np.float32)
    proj_w = np.asarray(proj_w, np.float32)
    proj_b = np.asarray(proj_b, np.float32)
    bias_idxs = np.asarray(bias_idxs, np.int32)

    scale = np.float32(KD ** -0.5)

    # Fold LayerNorm affine into the QKV projection (exact):
    #   qkv = (xhat*nw + nb) @ W^T + b = xhat @ (W*nw)^T + (W@nb + b)
    w_eff = qkv_w * norm_w[None, :]
    b_eff = qkv_b + qkv_w @ norm_b

    # Reorder rows into [all q | all k | all v] blocks and fold the q scale.
    fidx = np.arange(H * (2 * KD + VD)).reshape(H, 3, KD)
    q_rows = fidx[:, 0, :].ravel()
    k_rows = fidx[:, 1, :].ravel()
    v_rows = fidx[:, 2, :].ravel()
    wq = w_eff[q_rows] * scale
    bq = b_eff[q_rows] * scale
    wk_ = w_eff[k_rows]
    bk = b_eff[k_rows]
    wv = w_eff[v_rows]
    bv = b_eff[v_rows]

    wqk = np.concatenate([wq, wk_], axis=0).T.astype(BF16)        # [384, 768]
    wv_t = wv.T.astype(BF16)                                      # [384, 384]
    wp_t = proj_w.T.astype(BF16)                                  # [384, 384]

    # Relative-position bias transposed to [tok_k, tok_q], tok_k padded
    # 196->256; the PE preloads it into the scores PSUM, and padded keys get
    # bias -30 so exp() kills their probability.
    attn_bias = attention_biases[:, bias_idxs]                    # [H, nq, mk]
    bT = np.transpose(attn_bias, (0, 2, 1))                       # [H, mk, nq]
    ebias = np.zeros((128, H * 392), np.float32)
    for h in range(H):
        hb = bT[h]
        pad = -30.0
        ebias[:, 392 * h:392 * h + 196] = hb[0:128, :]
        ebias[0:NB, 392 * h + 196:392 * (h + 1)] = hb[128:N, :]
        ebias[NB:128, 392 * h + 196:392 * (h + 1)] = pad
    ebias = ebias.astype(BF16)

    has_qk_bias = bool(np.any(bq) or np.any(bk))
    has_v_bias = bool(np.any(bv))

    key = (has_qk_bias, has_v_bias)
    if key not in _CACHE:
        _CACHE[key] = _build_program(has_qk_bias, has_v_bias)
    nc = _CACHE[key]

    idm = np.eye(128, dtype=np.float32).astype(BF16)
    shared = {
        "wqk": wqk, "wv": wv_t, "wp": wp_t, "eb": ebias, "idm": idm,
    }
    if has_qk_bias:
        shared["qkb"] = np.concatenate([bq, bk]).reshape(-1, 1).astype(np.float32)
    if has_v_bias:
        shared["vb"] = np.broadcast_to(bv[None, :], (128, DIM)).copy().astype(np.float32)

    xs = x.reshape(NCORES, WPC * N, DIM)
    xpad = np.zeros((NCORES, WPC * N + 64, DIM), np.float32)
    xpad[:, :WPC * N, :] = xs
    in_maps = [dict(shared, x=xpad[c]) for c in range(NCORES)]

    from concourse.bass_utils import run_bass_kernel_spmd
    trace = bool(int(os.environ.get("KERNEL_TRACE", "0")))
    res = run_bass_kernel_spmd(nc, in_maps, core_ids=list(range(NCORES)),
                               trace=trace)
    global LAST_EXEC_NS
    LAST_EXEC_NS = res.exec_time_ns

    nbench = int(os.environ.get("KERNEL_BENCH", "0"))
    if nbench:
        import time
        times = []
        for _ in range(nbench):
            t0 = time.perf_counter()
            run_bass_kernel_spmd(nc, in_maps, core_ids=list(range(NCORES)))
            times.append(time.perf_counter() - t0)
        print("bench wall times (s):", [f"{t:.3f}" for t in times])
        if LAST_EXEC_NS is None:
            LAST_EXEC_NS = int(min(times) * 1e9)

    out = np.concatenate([r["out"] for r in res.results], axis=0)
    out = out.reshape(B, N, DIM)
    if np.any(proj_b):
        out = out + proj_b
    return np.ascontiguousarray(out.astype(np.float32))


LAST_EXEC_NS = None


# revision 25
# speedup vs baseline: 15.9332x; 15.9332x over previous
"""Windowed attention block (LeViT-style) on 8 Trainium2 NeuronCores.

LayerNorm -> QKV -> per-head biased softmax attention -> output projection
for B=256 windows, N=196 tokens, DIM=384, 12 heads of dim 32.

Sharding: data-parallel over the window dim B — 32 windows per core, weights
replicated, no collectives. Each core runs an identical Bass/Tile program on
its shard; the host concatenates the 8 output shards.

Kernel strategy (per window):
 - LN token-major via bn_stats + exp(-0.5*ln(var+eps)), applied on ScalarE
   (norm_w/norm_b are folded into the QKV weights on the host).
 - xhat transposed to feature-major via PE transpose (bf16).
 - QKV computed feature-major for q,k (so per-head qT/kT are direct slices at
   32-aligned partitions) and token-major for v (AV's lhsT layout).
 - scoresT[h] = kT_h^T @ qT_h with 4 heads running concurrently in distinct
   32-row PE groups (tile_position row tiling, K=32).
 - probs = exp(scoresT) * exp(bias)  (bias table gathered+exp'd on host; the
   multiplicative form lets ScalarE do exp straight out of PSUM while
   VectorE/GpSimd apply the bias in bf16).
 - AV col-tiled: per head pair, v-columns at PE col groups 0-1 and a [128,32]
   ones block at col groups 2-3, so one PSUM bank accumulates both heads'
   outputs (rows 0:64) and their softmax sums replicated 32x (rows 64:128).
   The replicated sums make 1/sum a contiguous [64,196] reciprocal whose
   output needs no broadcast for the normalize multiply.
 - proj from the normalized feature-major outT, fp32 result copied and DMA'd.

Token dim padded 196->256 (zero k-columns + zero ebias rows) so both tok_k
chunks use all 128 partitions.
"""

import os
import sys
import numpy as np

sys.path.insert(0, "/root/.axon_site/_ro/trn_rl_repo")

import ml_dtypes

B, N, DIM = 256, 196, 384
H, KD, VD = 12, 32, 32
RES = 14
EPS = 1e-5
NCORES = 8
WPC = B // NCORES          # windows per core
NP = 256                   # padded token count (2 chunks of 128)
NB = N - 128               # 68 = second token chunk size

BF16 = ml_dtypes.bfloat16


def _build_bias_idxs():
    pts = [(i, j) for i in range(RES) for j in range(RES)]
    offs, idxs = {}, []
    for p1 in pts:
        for p2 in pts:
            o = (abs(p1[0] - p2[0]), abs(p1[1] - p2[1]))
            if o not in offs:
                offs[o] = len(offs)
            idxs.append(offs[o])
    return np.array(idxs, dtype=np.int32).reshape(N, N)


def _split_waits(nc, keep=1):
    """Hoist excess sem-waits into standalone single-wait NoOps.

    The walrus build here rejects instructions whose sync region carries more
    than ~2 sync commands; Tile attaches every required wait directly to the
    instruction (and its tail drain waits on every live proc). A chain of
    single-wait NoOps on the same engine immediately before the instruction
    is semantically identical (the engine's instruction stream blocks), so
    this rewrite preserves correctness.
    """
    from concourse import mybir
    counter = [0]

    def fresh():
        counter[0] += 1
        return f"I-waitsplit-{counter[0]}"

    for f in nc.m.functions:
        for blk in f.blocks:
            out, changed = [], False
            for inst in blk.instructions:
                si = inst.sync_info
                waits = list(si.on_wait) if si is not None and si.on_wait else []
                if len(waits) > keep:
                    changed = True
                    for wt in waits[:-keep]:
                        nop = mybir.InstNoOp(name=fresh(), ins=[], outs=[])
                        nop.engine = inst.engine
                        nop.sync_info = mybir.SyncInfo(on_wait=[wt], on_update=[])
                        out.append(nop)
                    inst.sync_info = mybir.SyncInfo(
                        on_wait=waits[-keep:],
                        on_update=list(si.on_update) if si.on_update else [])
                out.append(inst)
            if changed:
                blk.instructions = out


def _build_program(has_qk_bias, has_v_bias):
    import concourse.bass as bass
    import concourse.tile as tile
    from concourse import mybir

    F32 = mybir.dt.float32
    BF = mybir.dt.bfloat16
    AF = mybir.ActivationFunctionType
    ALU = mybir.AluOpType

    nc = bass.Bass()
    # Pre-register an eps const AP so `activation(..., bias=EPS)` carries no
    # runtime dependency (mirrors Bass's own const-AP registration).
    _epsc = nc.alloc_sbuf_tensor("const-eps", [128, 1], F32)
    nc.gpsimd.memset(_epsc.ap(), EPS)
    nc.const_aps.aps[(F32, float(EPS))] = _epsc.ap()
    nc.all_engine_barrier()
    # x is stored flat with 64 rows of zero padding so each window can be
    # fetched as one [128, 768] interleaved DMA (token t -> partition t%128,
    # column block t//128).
    x_d = nc.dram_tensor("x", [WPC * N + 64, DIM], F32, kind="ExternalInput")
    wqk_d = nc.dram_tensor("wqk", [DIM, 2 * DIM], BF, kind="ExternalInput")
    wv_d = nc.dram_tensor("wv", [DIM, DIM], BF, kind="ExternalInput")
    wp_d = nc.dram_tensor("wp", [DIM, DIM], BF, kind="ExternalInput")
    # raw (transposed) attention bias per head, preloaded into the scores
    # PSUM bank via an identity matmul before the score matmuls accumulate.
    eb_d = nc.dram_tensor("eb", [128, H * 392], BF, kind="ExternalInput")
    id_d = nc.dram_tensor("idm", [128, 128], BF, kind="ExternalInput")
    if has_qk_bias:
        qkb_d = nc.dram_tensor("qkb", [2 * DIM, 1], F32, kind="ExternalInput")
    if has_v_bias:
        vb_d = nc.dram_tensor("vb", [128, DIM], F32, kind="ExternalInput")
    out_d = nc.dram_tensor("out", [WPC, N, DIM], F32, kind="ExternalOutput")

    with tile.TileContext(nc) as tc:
        with tc.tile_pool(name="const", bufs=1) as cp, \
             tc.tile_pool(name="work", bufs=2) as wk, \
             tc.tile_pool(name="ps", bufs=8, space="PSUM") as ps:

            # ---- persistent constants ----
            wqk_sb = []
            for i in range(3):
                t = cp.tile([128, 2 * DIM], BF, name=f"wqk{i}")
                nc.sync.dma_start(t, wqk_d.ap()[128 * i:128 * (i + 1), :])
                wqk_sb.append(t)
            wv_sb = []
            for i in range(3):
                t = cp.tile([128, DIM], BF, name=f"wv{i}")
                nc.sync.dma_start(t, wv_d.ap()[128 * i:128 * (i + 1), :])
                wv_sb.append(t)
            wp_sb = []
            for i in range(3):
                t = cp.tile([128, DIM], BF, name=f"wp{i}")
                nc.sync.dma_start(t, wp_d.ap()[128 * i:128 * (i + 1), :])
                wp_sb.append(t)
            eb_sb = cp.tile([128, H * 392], BF, name="ebias")
            nc.sync.dma_start(eb_sb, eb_d.ap())
            ident = cp.tile([128, 128], BF, name="ident")
            nc.sync.dma_start(ident, id_d.ap())
            ones_sb = cp.tile([128, 32], BF, name="ones32")
            nc.gpsimd.memset(ones_sb, 1.0)
            if has_qk_bias:
                qkb_sb = []
                for i in range(6):
                    t = cp.tile([128, 1], F32, name=f"qkb{i}")
                    nc.sync.dma_start(t, qkb_d.ap()[128 * i:128 * (i + 1), :])
                    qkb_sb.append(t)
            if has_v_bias:
                vb_sb = cp.tile([128, DIM], F32, name="vbias")
                nc.sync.dma_start(vb_sb, vb_d.ap())

            for wp in range(WPC // 2):
                # Two windows are processed jointly through the feature-major
                # stages (transpose/q/k), doubling those tiles' free dim to
                # 392 and halving the op count on the copy-bound engines.
                toks = [(0, 128), (128, NB)]
                wins = (2 * wp, 2 * wp + 1)
                xh_w = []
                v_w = []
                for sub, w in enumerate(wins):
                    # ---- load x: one interleaved DMA [128, 768] ----
                    x_sb = wk.tile([128, 2 * DIM], F32, name=f"x{sub}")
                    nc.sync.dma_start(
                        x_sb,
                        bass.AP(x_d, w * N * DIM,
                                [[DIM, 128], [128 * DIM, 2], [1, DIM]]))

                    # ---- LayerNorm (stats DVE, ln/exp ACT, apply GpSimd) ----
                    xh_t = []
                    for ci, (t0, tn) in enumerate(toks):
                        xc = x_sb[:, DIM * ci:DIM * (ci + 1)]
                        bn6 = wk.tile([128, 6], F32, name=f"bn{sub}{ci}")
                        nc.vector.bn_stats(bn6, xc)
                        mv = wk.tile([128, 2], F32, name=f"mv{sub}{ci}")
                        nc.vector.bn_aggr(mv, bn6)
                        lnv = wk.tile([128, 1], F32, name=f"lnv{sub}{ci}")
                        nc.scalar.activation(lnv, mv[:, 1:2], AF.Ln, bias=EPS)
                        rstd = wk.tile([128, 1], F32, name=f"rstd{sub}{ci}")
                        nc.scalar.activation(rstd, lnv, AF.Exp, scale=-0.5)
                        nrstd = wk.tile([128, 1], F32, name=f"nrstd{sub}{ci}")
                        nc.scalar.activation(nrstd, rstd, AF.Identity, scale=-1.0)
                        nmr = wk.tile([128, 1], F32, name=f"nmr{sub}{ci}")
                        nc.scalar.activation(nmr, mv[:, 0:1], AF.Identity,
                                             scale=nrstd[:, 0:1])
                        xh = wk.tile([128, DIM], BF, name=f"xh{sub}{ci}")
                        nc.gpsimd.tensor_scalar(xh, xc, rstd[:, 0:1], nmr[:, 0:1],
                                                ALU.mult, ALU.add)
                        xh_t.append(xh)
                    xh_w.append(xh_t)

                # ---- transpose xhat -> feature-major [384, 392] (3 tiles) ----
                xT = []
                for i in range(3):
                    xT_ps = ps.tile([128, 2 * N], BF, name="xTps", tag="bank")
                    for sub in range(2):
                        o = N * sub
                        nc.tensor.transpose(xT_ps[:, o:o + 128],
                                            xh_w[sub][0][:, 128 * i:128 * (i + 1)],
                                            ident)
                        nc.tensor.transpose(xT_ps[:, o + 128:o + N],
                                            xh_w[sub][1][0:NB, 128 * i:128 * (i + 1)],
                                            ident[0:NB, 0:NB])
                    xTs = wk.tile([128, 2 * N], BF, name=f"xT{i}", bufs=6)
                    nc.vector.tensor_copy(xTs, xT_ps)
                    xT.append(xTs)

                # ---- q,k feature-major for both windows ----
                q_sb, k_sb = [], []
                for i in range(3):
                    qp = ps.tile([128, 2 * N], F32, name="qps", tag="bank")
                    for d in range(3):
                        nc.tensor.matmul(qp, wqk_sb[d][:, 128 * i:128 * (i + 1)],
                                         xT[d], start=(d == 0), stop=(d == 2))
                    qs = wk.tile([128, 2 * N], BF, name=f"q{i}", bufs=6)
                    if has_qk_bias:
                        nc.scalar.activation(qs, qp, AF.Identity,
                                             bias=qkb_sb[i][:, 0:1])
                    else:
                        nc.scalar.copy(qs, qp)
                    q_sb.append(qs)
                for i in range(3):
                    kp = ps.tile([128, 2 * N], F32, name="kps", tag="bank")
                    for d in range(3):
                        nc.tensor.matmul(kp, wqk_sb[d][:, DIM + 128 * i:DIM + 128 * (i + 1)],
                                         xT[d], start=(d == 0), stop=(d == 2))
                    # layout [w0 196 | pad 60 | w1 196 | pad 60]
                    ks = wk.tile([128, 2 * NP], BF, name=f"k{i}", bufs=6)
                    nc.gpsimd.memset(
                        bass.AP(ks.tensor, ks.offset + N,
                                [list(ks.ap[0]), [NP, 2], [1, NP - N]]), 0.0)
                    dst = bass.AP(ks.tensor, ks.offset,
                                  [list(ks.ap[0]), [NP, 2], [1, N]])
                    if has_qk_bias:
                        nc.scalar.activation(dst, kp, AF.Identity,
                                             bias=qkb_sb[3 + i][:, 0:1])
                    else:
                        nc.vector.tensor_copy(dst, kp)
                    k_sb.append(ks)

                # ---- v token-major [256(pad), 384] per window ----
                for sub in range(2):
                    v_sb = []
                    for ci, (t0, tn) in enumerate(toks):
                        vp = ps.tile([128, DIM], F32, name="vps", tag="bank")
                        for d in range(3):
                            nc.tensor.matmul(vp[0:tn, :],
                                             xT[d][:, N * sub + t0:N * sub + t0 + tn],
                                             wv_sb[d], start=(d == 0), stop=(d == 2))
                        vs = wk.tile([128, DIM], BF, name=f"v{sub}{ci}", bufs=4)
                        if tn < 128:
                            # pad rows zeroed; the copy below rewrites real
                            # rows 64:tn (Tile orders the overlapping writes)
                            nc.gpsimd.memset(vs[64:128, :], 0.0)
                        nc.vector.tensor_copy(vs[0:tn, :], vp[0:tn, :])
                        if has_v_bias:
                            nc.vector.tensor_tensor(vs[0:tn, :], vs[0:tn, :],
                                                    vb_sb[0:tn, :], ALU.add)
                        v_sb.append(vs)
                    v_w.append(v_sb)

                for sub, w in enumerate(wins):
                    qo, ko = N * sub, NP * sub
                    v_sb = v_w[sub]
                    # ---- scoresT + probs per head ----
                    # The (transposed) attention bias is preloaded into the
                    # scores PSUM bank by an identity matmul; the two K=32
                    # row-tiled score matmuls accumulate on top, so exp(PSUM)
                    # IS the probability tile (padded keys: bias -30 -> ~0).
                    probs2 = [None] * H
                    for g in range(3):
                        # all 4 bias preloads (full-array matmuls) first, so
                        # the 8 K=32 score matmuls that follow can overlap in
                        # their four distinct PE row groups
                        sps = []
                        for j in range(4):
                            h = 4 * g + j
                            sp = ps.tile([128, 392], F32, name="scp",
                                         tag="bank")
                            nc.tensor.matmul(sp, ident,
                                             eb_sb[:, 392 * h:392 * (h + 1)],
                                             start=True, stop=False,
                                             skip_group_check=True)
                            sps.append(sp)
                        for j in range(4):
                            r = 32 * j
                            nc.tensor.matmul(sps[j][:, 0:N],
                                             k_sb[g][r:r + 32, ko:ko + 128],
                                             q_sb[g][r:r + 32, qo:qo + N],
                                             tile_position=(r, 0),
                                             start=False, stop=False,
                                             skip_group_check=True)
                        for j in range(4):
                            r = 32 * j
                            nc.tensor.matmul(sps[j][:, 196:196 + N],
                                             k_sb[g][r:r + 32, ko + 128:ko + NP],
                                             q_sb[g][r:r + 32, qo:qo + N],
                                             tile_position=(r, 0),
                                             start=False, stop=True,
                                             skip_group_check=True)
                        for j in range(4):
                            h = 4 * g + j
                            pr = wk.tile([128, 392], BF, name="probs", bufs=14)
                            nc.scalar.activation(pr, sps[j], AF.Exp)
                            probs2[h] = pr

                    # ---- AV + sums per pair; reciprocal of replicated sums ----
                    av_ps, rc_sb = [], []
                    for p in range(6):
                        h0, h1 = 2 * p, 2 * p + 1
                        ap_ = ps.tile([128, N], F32, name="avp", tag="bank")
                        for c in range(2):
                            st, fi = (c == 0), (c == 1)
                            pa = probs2[h0][:, 196 * c:196 * (c + 1)]
                            pb = probs2[h1][:, 196 * c:196 * (c + 1)]
                            nc.tensor.matmul(ap_[0:32, :],
                                             v_sb[c][:, 32 * h0:32 * h0 + 32],
                                             pa, start=st, stop=fi,
                                             tile_position=(0, 0))
                            nc.tensor.matmul(ap_[32:64, :],
                                             v_sb[c][:, 32 * h1:32 * h1 + 32],
                                             pb, start=st, stop=fi,
                                             tile_position=(0, 32))
                            nc.tensor.matmul(ap_[64:96, :], ones_sb, pa,
                                             start=st, stop=fi,
                                             tile_position=(0, 64))
                            nc.tensor.matmul(ap_[96:128, :], ones_sb, pb,
                                             start=st, stop=fi,
                                             tile_position=(0, 96))
                        rc = wk.tile([64, N], F32, name="rc", bufs=8)
                        nc.vector.reciprocal(rc, ap_[64:128, :])
                        av_ps.append(ap_)
                        rc_sb.append(rc)

                    # ---- normalize -> outT feature-major [384, 196] bf16 ----
                    oT = []
                    for t in range(3):
                        o = wk.tile([128, N], BF, name=f"oT{t}", bufs=6)
                        nc.vector.tensor_tensor(o[0:64, :], av_ps[2 * t][0:64, :],
                                                rc_sb[2 * t], ALU.mult)
                        nc.vector.tensor_tensor(o[64:128, :],
                                                av_ps[2 * t + 1][0:64, :],
                                                rc_sb[2 * t + 1], ALU.mult)
                        oT.append(o)

                    # ---- projection + store ----
                    for ci, (t0, tn) in enumerate(toks):
                        pp = ps.tile([128, DIM], F32, name="pjps", tag="bank")
                        for t in range(3):
                            nc.tensor.matmul(pp[0:tn, :], oT[t][:, t0:t0 + tn],
                                             wp_sb[t], start=(t == 0), stop=(t == 2))
                        ob = wk.tile([128, DIM], F32, name=f"ob{ci}")
                        nc.scalar.copy(ob[0:tn, :], pp[0:tn, :])
                        # store via gpsimd SWDGE to keep the SP sequencer free
                        nc.gpsimd.dma_start(out_d.ap()[w, t0:t0 + tn, :],
                                            ob[0:tn, :])

    _split_waits(nc)
    return nc


_CACHE = {}


def kernel(x, norm_w, norm_b, qkv_w, qkv_b, attention_biases, proj_w, proj_b,
           bias_idxs):
    x = np.asarray(x, np.float32)
    norm_w = np.asarray(norm_w, np.float32)
    norm_b = np.asarray(norm_b, np.float32)
    qkv_w = np.asarray(qkv_w, np.float32)
    qkv_b = np.asarray(qkv_b, np.float32)
    attention_biases = np.asarray(attention_biases, np.float32)
    proj_w = np.asarray(proj_w, np.float32)
    proj_b = np.asarray(proj_b, np.float32)
    bias_idxs = np.asarray(bias_idxs, np.int32)

    scale = np.float32(KD ** -0.5)

    # Fold LayerNorm affine into the QKV projection (exact):
    #   qkv = (xhat*nw + nb) @ W^T + b = xhat @ (W*nw)^T + (W@nb + b)
    w_eff = qkv_w * norm_w[None, :]
    b_eff = qkv_b + qkv_w @ norm_b

    # Reorder rows into [all q | all k | all v] blocks and fold the q scale.
    fidx = np.arange(H * (2 * KD + VD)).reshape(H, 3, KD)
    q_rows = fidx[:, 0, :].ravel()
    k_rows = fidx[:, 1, :].ravel()
    v_rows = fidx[:, 2, :].ravel()
    wq = w_eff[q_rows] * scale
    bq = b_eff[q_rows] * scale
    wk_ = w_eff[k_rows]
    bk = b_eff[k_rows]
    wv = w_eff[v_rows]
    bv = b_eff[v_rows]

    wqk = np.concatenate([wq, wk_], axis=0).T.astype(BF16)        # [384, 768]
    wv_t = wv.T.astype(BF16)                                      # [384, 384]
    wp_t = proj_w.T.astype(BF16)                                  # [384, 384]

    # Relative-position bias transposed to [tok_k, tok_q], tok_k padded
    # 196->256; the PE preloads it into the scores PSUM, and padded keys get
    # bias -30 so exp() kills their probability.
    attn_bias = attention_biases[:, bias_idxs]                    # [H, nq, mk]
    bT = np.transpose(attn_bias, (0, 2, 1))                       # [H, mk, nq]
    ebias = np.zeros((128, H * 392), np.float32)
    for h in range(H):
        hb = bT[h]
        pad = -30.0
        ebias[:, 392 * h:392 * h + 196] = hb[0:128, :]
        ebias[0:NB, 392 * h + 196:392 * (h + 1)] = hb[128:N, :]
        ebias[NB:128, 392 * h + 196:392 * (h + 1)] = pad
    ebias = ebias.astype(BF16)

    has_qk_bias = bool(np.any(bq) or np.any(bk))
    has_v_bias = bool(np.any(bv))

    key = (has_qk_bias, has_v_bias)
    if key not in _CACHE:
        _CACHE[key] = _build_program(has_qk_bias, has_v_bias)
    nc = _CACHE[key]

    idm = np.eye(128, dtype=np.float32).astype(BF16)
    shared = {
        "wqk": wqk, "wv": wv_t, "wp": wp_t, "eb": ebias, "idm": idm,
    }
    if has_qk_bias:
        shared["qkb"] = np.concatenate([bq, bk]).reshape(-1, 1).astype(np.float32)
    if has_v_bias:
        shared["vb"] = np.broadcast_to(bv[None, :], (128, DIM)).copy().astype(np.float32)

    xs = x.reshape(NCORES, WPC * N, DIM)
    xpad = np.zeros((NCORES, WPC * N + 64, DIM), np.float32)
    xpad[:, :WPC * N, :] = xs
    in_maps = [dict(shared, x=xpad[c]) for c in range(NCORES)]

    from concourse.bass_utils import run_bass_kernel_spmd
    trace = bool(int(os.environ.get("KERNEL_TRACE", "0")))
    res = run_bass_kernel_spmd(nc, in_maps, core_ids=list(range(NCORES)),
                               trace=trace)
    global LAST_EXEC_NS
    LAST_EXEC_NS = res.exec_time_ns

    nbench = int(os.environ.get("KERNEL_BENCH", "0"))
    if nbench:
        import time
        times = []
        for _ in range(nbench):
            t0 = time.perf_counter()
            run_bass_kernel_spmd(nc, in_maps, core_ids=list(range(NCORES)))
            times.append(time.perf_counter() - t0)
        print("bench wall times (s):", [f"{t:.3f}" for t in times])
        if LAST_EXEC_NS is None:
            LAST_EXEC_NS = int(min(times) * 1e9)

    out = np.concatenate([r["out"] for r in res.results], axis=0)
    out = out.reshape(B, N, DIM)
    if np.any(proj_b):
        out = out + proj_b
    return np.ascontiguousarray(out.astype(np.float32))


LAST_EXEC_NS = None
